# revision 15
# baseline (speedup 1.0000x reference)
"""Transformer encoder layer (post-norm, 16 heads, d_model=1024, d_ff=4096)
on 8 Trainium2 NeuronCores.

Sharding: batch(4) x seq-half(2) -> 8 shards. Each core computes K/V for its
batch's FULL sequence and Q/attention/FFN/LN for its 1024-query half.

v2: software-pipelined. The softmax exp (ACT engine) is the attention
bottleneck; the kernel interleaves the second half of the V-projection into
query-block j0's attention and all of O-proj/LN1/FFN for j0 into j1's
attention, so the PE has dense work while ACT churns through exp. Scores
use 64x128 row-tiled matmuls (two heads run concurrently in the PE array,
K=64 each, no zero padding); exp runs on 2-bank PSUM groups ([P,2,512] per
instruction) to amortize ACT access overhead. attn@V accumulates 8
independent [128,65] regions across two single-bank PSUM tiles (one
start/stop pair per bank). All PE transposes are bf16. Softmax skips
max-subtraction (scores ~ N(0,1)) - identical after normalization.
"""

import numpy as np
import ml_dtypes

B, S, D = 4, 2048, 1024
H, DK = 16, 64
DFF = 4096
SQ = S // 2          # queries per core
P = 128              # partitions
EPS = 1e-6
NCORES = 8

BF16 = ml_dtypes.bfloat16

_PROG = None  # cached compiled program


def _build_program():
    import concourse.bacc as bacc
    import concourse.tile as tile
    import concourse.mybir as mybir
    from concourse.masks import make_identity

    f32 = mybir.dt.float32
    bf16 = mybir.dt.bfloat16
    AF = mybir.ActivationFunctionType
    Alu = mybir.AluOpType

    nc = bacc.Bacc("TRN2", target_bir_lowering=False, debug=False,
                   num_devices=NCORES)

    # ---- DRAM parameters (per-core shards supplied by host) ----
    xt = nc.declare_dram_parameter("xt", [D, S], bf16, isOutput=False)    # x[b].T
    xh = nc.declare_dram_parameter("xh", [SQ, D], bf16, isOutput=False)   # x_half + bo
    wq = nc.declare_dram_parameter("wq", [D, D], bf16, isOutput=False)
    wk = nc.declare_dram_parameter("wk", [D, D], bf16, isOutput=False)
    wv = nc.declare_dram_parameter("wv", [D, D], bf16, isOutput=False)
    wo = nc.declare_dram_parameter("wo", [D, D], bf16, isOutput=False)
    w1 = nc.declare_dram_parameter("w1", [D, DFF], bf16, isOutput=False)
    w2 = nc.declare_dram_parameter("w2", [DFF, D], bf16, isOutput=False)
    bq = nc.declare_dram_parameter("bq", [D], f32, isOutput=False)
    bk = nc.declare_dram_parameter("bk", [D], f32, isOutput=False)
    bvh = nc.declare_dram_parameter("bvh", [D], bf16, isOutput=False)
    b1p = nc.declare_dram_parameter("b1", [DFF], f32, isOutput=False)
    a1p = nc.declare_dram_parameter("alpha1", [D], bf16, isOutput=False)
    g1p = nc.declare_dram_parameter("beta1", [D], bf16, isOutput=False)
    a2p = nc.declare_dram_parameter("alpha2", [D], bf16, isOutput=False)
    g2p = nc.declare_dram_parameter("beta2", [D], bf16, isOutput=False)
    out = nc.declare_dram_parameter("out", [SQ, D], f32, isOutput=True)

    KC = D // P          # 8 k-chunks of 128
    DCH = D // P         # 8 feature chunks
    SCH = S // P         # 16 s-chunks
    NW = 512

    import concourse.bass as bass

    def bcast(ap_1d, n):
        return bass.AP(tensor=ap_1d.tensor, offset=ap_1d.offset,
                       ap=[[0, P]] + list(ap_1d.ap[-1:]))[:, 0:n]

    with tile.TileContext(nc) as tc:
        with tc.tile_pool(name="main", bufs=1) as mp, \
             tc.tile_pool(name="wp", bufs=2) as wp, \
             tc.tile_pool(name="etp", bufs=4) as etp, \
             tc.tile_pool(name="at2p", bufs=4) as at2p, \
             tc.tile_pool(name="small", bufs=4) as smp, \
             tc.tile_pool(name="tokp", bufs=1) as tokp, \
             tc.tile_pool(name="outp", bufs=1) as outp, \
             tc.tile_pool(name="scp", bufs=2, space="PSUM") as scp, \
             tc.tile_pool(name="patp", bufs=1, space="PSUM") as patp, \
             tc.tile_pool(name="trp", bufs=1, space="PSUM") as trp, \
             tc.tile_pool(name="miscp", bufs=1, space="PSUM") as miscp:

            # ---- constants ----
            ident_bf = mp.tile([P, P], bf16, tag="ident_bf")
            make_identity(nc, ident_bf)

            bq_sb = mp.tile([P, DCH], f32, tag="bq")
            nc.sync.dma_start(out=bq_sb, in_=bq[:].rearrange("(c p) -> p c", p=P))
            bk_sb = mp.tile([P, DCH], f32, tag="bk")
            nc.sync.dma_start(out=bk_sb, in_=bk[:].rearrange("(c p) -> p c", p=P))
            b1_sb = mp.tile([P, DFF // P], f32, tag="b1")
            nc.sync.dma_start(out=b1_sb, in_=b1p[:].rearrange("(c p) -> p c", p=P))
            bv_b = mp.tile([P, NW], bf16, tag="bv_b")
            nc.sync.dma_start(out=bv_b, in_=bcast(bvh[:], D)[:, 0:NW])
            a1_b = mp.tile([P, D], bf16, tag="a1_b")
            nc.sync.dma_start(out=a1_b, in_=bcast(a1p[:], D))
            g1_b = mp.tile([P, D], bf16, tag="g1_b")
            nc.sync.dma_start(out=g1_b, in_=bcast(g1p[:], D))
            a2_b = mp.tile([P, D], bf16, tag="a2_b")
            nc.sync.dma_start(out=a2_b, in_=bcast(a2p[:], D))
            g2_b = mp.tile([P, D], bf16, tag="g2_b")
            nc.sync.dma_start(out=g2_b, in_=bcast(g2p[:], D))

            # prepay the exp ACT table load
            warm = mp.tile([P, 1], f32, tag="warm")
            nc.vector.memset(warm, 0.0)
            nc.scalar.activation(warm, warm, AF.Exp)

            # ---- persistent SBUF tensors ----
            # slotA: xtb -> relu_j0 ; slotC: qtb -> relu_j1
            xtb = mp.tile([P, KC, S], bf16, tag="slotA")
            for xh_ in range(2):
                nc.sync.dma_start(
                    out=xtb[:, :, xh_ * SQ:(xh_ + 1) * SQ],
                    in_=xt[:, xh_ * SQ:(xh_ + 1) * SQ].rearrange(
                        "(c p) s -> p c s", p=P))

            ktb = mp.tile([P, DCH, S], bf16, tag="slotB")
            qtb = mp.tile([P, H, SQ], bf16, tag="slotC")
            vaug = mp.tile([P, SCH, H * (DK + 1)], bf16, tag="slotD")
            va_view = vaug.rearrange("p s (h w) -> p s h w", w=DK + 1)
            nc.vector.memset(va_view[:, :, :, DK:DK + 1], 1.0)

            # j0's O-proj is fully front-loaded into j1's first attention
            # pair, so concat-j0 is dead before concat-j1's first write:
            # one shared buffer (WAR tracked by Tile).
            cc_t = mp.tile([P, DCH, NW], bf16, tag="cc")
            concat = [cc_t, cc_t]
            norm1r = mp.tile([P, 4, D], bf16, tag="n1r")     # affined residual
            norm1T = mp.tile([P, DCH, NW], bf16, tag="n1T")  # z transposed
            s2t = mp.tile([P, 4, D], bf16, tag="s2t")        # ffn2 + residual

            # weight chunk loader: [P, KC, 512] slices (8 KB each)
            def wload(src, col0):
                t = wp.tile([P, KC, NW], bf16, tag="w")
                nc.sync.dma_start(
                    out=t, in_=src[:, col0:col0 + NW].rearrange(
                        "(c p) n -> p c n", p=P))
                return t

            def w2load(k0, col0):
                t = wp.tile([P, 16, 256], bf16, tag="w")
                nc.sync.dma_start(
                    out=t, in_=w2[k0 * P:(k0 + 16) * P, col0:col0 + 256]
                    .rearrange("(c p) n -> p c n", p=P))
                return t

            wv_c = [None, None]
            wo_c = [None, None]
            w1_c = [None] * 8
            w2_c = [None, None]

            # misc psum provider: in-window -> single miscp bank;
            # tail -> rotate through scp banks (free then) to avoid WAR stalls
            misc_state = {"tail": False, "i": 0, "tile": None}

            def misc_bank():
                if not misc_state["tail"]:
                    return miscp.tile([P, NW], f32, tag="m", name="mb")
                i = misc_state["i"]
                if i % 2 == 0:
                    misc_state["tile"] = scp.tile([P, 2, NW], f32, tag="sc",
                                                  name="mb2")
                misc_state["i"] = i + 1
                return misc_state["tile"][:, i % 2, :]

            # ================= K projection =================
            with nc.named_scope("qkv"):
                wk_c = [wload(wk, 0), wload(wk, NW)]
                wq_c = [wload(wq, 0), wload(wq, NW)]
                for dch in range(DCH):
                    wch = wk_c[dch // 4]
                    wcol = (dch % 4) * P
                    for half in range(2):
                        st = scp.tile([P, 2, NW], f32, tag="sc")
                        for n2 in range(2):
                            for kc in range(KC):
                                nc.tensor.matmul(
                                    st[:, n2, :],
                                    wch[:, kc, wcol:wcol + P],
                                    xtb[:, kc, (half * 2 + n2) * NW:
                                        (half * 2 + n2 + 1) * NW],
                                    start=(kc == 0), stop=(kc == KC - 1))
                        nc.scalar.activation(
                            ktb[:, dch, half * 1024:(half + 1) * 1024]
                            .rearrange("p (a b) -> p a b", a=2),
                            st, AF.Identity, bias=bk_sb[:, dch:dch + 1])

                # ================= Q projection =================
                wv_c[0] = wload(wv, 0)
                wv_c[1] = wload(wv, NW)
                for dch in range(DCH):
                    wch = wq_c[dch // 4]
                    wcol = (dch % 4) * P
                    st = scp.tile([P, 2, NW], f32, tag="sc")
                    for n2 in range(2):
                        for kc in range(KC):
                            nc.tensor.matmul(
                                st[:, n2, :],
                                wch[:, kc, wcol:wcol + P],
                                xtb[:, kc, n2 * NW:(n2 + 1) * NW],
                                start=(kc == 0), stop=(kc == KC - 1))
                    stf = st.rearrange("p a b -> p (a b)")
                    nc.scalar.activation(
                        qtb[0:64, 2 * dch, :], stf[0:64, :],
                        AF.Identity, bias=bq_sb[0:64, dch:dch + 1])
                    nc.scalar.activation(
                        qtb[64:128, 2 * dch + 1, :], stf[64:128, :],
                        AF.Identity, bias=bq_sb[64:128, dch:dch + 1])

                # V projection, heads 0-7 (n=0), all sequence chunks, upfront
                def v_group(sch, n):
                    pt = misc_bank()
                    for kc in range(KC):
                        nc.tensor.matmul(
                            pt, xtb[:, kc, sch * P:(sch + 1) * P],
                            wv_c[n][:, kc, :],
                            start=(kc == 0), stop=(kc == KC - 1))
                    h0 = n * (NW // DK)
                    nc.vector.tensor_add(
                        va_view[:, sch, h0:h0 + 8, 0:DK],
                        pt.rearrange("p (h w) -> p h w", w=DK),
                        bv_b.rearrange("p (h w) -> p h w", w=DK))

                for sch in range(SCH):
                    v_group(sch, 0)

            # ---------- O-proj + LN1 + FFN closures for query-block j ------
            relu_j = [None, None]

            def ln_sqrt_rec(mv, corr):
                std_t = smp.tile([P, 1], f32, tag="std")
                rec_t = smp.tile([P, 1], f32, tag="recs")
                mean_t = smp.tile([P, 1], f32, tag="mean")
                nc.scalar.activation(std_t, mv[:, 1:2], AF.Sqrt,
                                     scale=float(corr))
                nc.vector.tensor_scalar_add(std_t, std_t, float(EPS))
                nc.vector.reciprocal(rec_t, std_t)
                nc.vector.tensor_copy(mean_t, mv[:, 0:1])
                return mean_t, rec_t

            def offn_closures(j):
                """64 closures: O+add(8), LN1(4), FFN1(32), FFN2(16), LN2(4)."""
                cls = []
                n1f = [None]

                def o_group(sq, n):
                    if sq == 0 and n == 0:
                        wo_c[0] = wload(wo, 0)
                        wo_c[1] = wload(wo, NW)
                    pt = misc_bank()
                    for kc in range(KC):
                        nc.tensor.matmul(
                            pt, concat[j][:, kc, sq * P:(sq + 1) * P],
                            wo_c[n][:, kc, :],
                            start=(kc == 0), stop=(kc == KC - 1))
                    if n == 0:
                        xh_t = tokp.tile([P, D], bf16, tag="tok")
                        nc.sync.dma_start(
                            out=xh_t,
                            in_=xh[(j * 4 + sq) * P:(j * 4 + sq + 1) * P, :])
                        n1f[0] = (outp.tile([P, D], f32, tag="of",
                                            name="n1f"), xh_t)
                    s1, xh_t = n1f[0]
                    nc.vector.tensor_add(
                        s1[:, n * NW:(n + 1) * NW], pt,
                        xh_t[:, n * NW:(n + 1) * NW])

                def ln1(sq):
                    s1, _ = n1f[0]
                    stats = smp.tile([P, 2, 6], f32, tag="stats")
                    nc.vector.bn_stats(stats[:, 0, :], s1[:, 0:NW])
                    nc.vector.bn_stats(stats[:, 1, :], s1[:, NW:D])
                    mv = smp.tile([P, 2], f32, tag="mv")
                    nc.vector.bn_aggr(mv, stats)
                    mean_t, rec_t = ln_sqrt_rec(mv, D / (D - 1))
                    zb = smp.tile([P, D], bf16, tag="zb", bufs=1)
                    nc.vector.tensor_scalar(
                        zb, s1, mean_t, rec_t, op0=Alu.subtract, op1=Alu.mult)
                    # affined residual for FFN2 (beta1 has b2 folded host-side)
                    nc.gpsimd.tensor_mul(norm1r[:, sq, :], zb, a1_b)
                    nc.gpsimd.tensor_add(norm1r[:, sq, :], norm1r[:, sq, :],
                                         g1_b)
                    # transpose plain z -> norm1T (alpha1 folded into W1)
                    for dch in range(DCH):
                        ptr = trp.tile([P, P], bf16, tag="tr")
                        nc.tensor.transpose(
                            ptr, zb[:, dch * P:(dch + 1) * P], ident_bf)
                        nc.vector.tensor_copy(
                            norm1T[:, dch, sq * P:(sq + 1) * P], ptr)

                def ffn1(t):
                    if t % 4 == 0:
                        w1_c[t // 4] = wload(w1, t * P)
                    wch = w1_c[t // 4]
                    pt = misc_bank()
                    for kc in range(KC):
                        nc.tensor.matmul(
                            pt, wch[:, kc, (t % 4) * P:(t % 4 + 1) * P],
                            norm1T[:, kc, :],
                            start=(kc == 0), stop=(kc == KC - 1))
                    nc.scalar.activation(relu_j[j][:, t, :], pt, AF.Relu,
                                         bias=b1_sb[:, t:t + 1])

                def ffn2(ncol, sq):
                    if sq == 0:
                        w2_c[0] = w2load(0, ncol * 256)
                        w2_c[1] = w2load(16, ncol * 256)
                    pt = misc_bank()[:, 0:256]
                    for kc in range(DFF // P):
                        nc.tensor.matmul(
                            pt, relu_j[j][:, kc, sq * P:(sq + 1) * P],
                            w2_c[kc // 16][:, kc % 16, :],
                            start=(kc == 0), stop=(kc == DFF // P - 1))
                    nc.vector.tensor_add(
                        s2t[:, sq, ncol * 256:(ncol + 1) * 256], pt,
                        norm1r[:, sq, ncol * 256:(ncol + 1) * 256])

                def ln2(sq):
                    s2 = s2t[:, sq, :]
                    stats = smp.tile([P, 2, 6], f32, tag="stats")
                    nc.vector.bn_stats(stats[:, 0, :], s2[:, 0:NW])
                    nc.vector.bn_stats(stats[:, 1, :], s2[:, NW:D])
                    mv = smp.tile([P, 2], f32, tag="mv")
                    nc.vector.bn_aggr(mv, stats)
                    mean_t, rec_t = ln_sqrt_rec(mv, D / (D - 1))
                    of = outp.tile([P, D], f32, tag="of")
                    nc.vector.tensor_scalar(
                        of, s2, mean_t, rec_t, op0=Alu.subtract, op1=Alu.mult)
                    nc.gpsimd.tensor_mul(of[:, 0:NW], of[:, 0:NW],
                                         a2_b[:, 0:NW])
                    nc.vector.tensor_mul(of[:, NW:D], of[:, NW:D],
                                         a2_b[:, NW:D])
                    nc.gpsimd.tensor_add(of[:, 0:NW], of[:, 0:NW],
                                         g2_b[:, 0:NW])
                    nc.vector.tensor_add(of[:, NW:D], of[:, NW:D],
                                         g2_b[:, NW:D])
                    r0 = (j * 4 + sq) * P
                    nc.sync.dma_start(out=out[r0:r0 + P, :], in_=of)

                for sq in range(4):
                    cls.append(lambda sq=sq: o_group(sq, 0))
                    cls.append(lambda sq=sq: o_group(sq, 1))
                    cls.append(lambda sq=sq: ln1(sq))
                for t in range(32):
                    cls.append(lambda t=t: ffn1(t))
                for ncol in range(4):
                    for sq in range(4):
                        cls.append(lambda ncol=ncol, sq=sq: ffn2(ncol, sq))
                for sq in range(4):
                    cls.append(lambda sq=sq: ln2(sq))
                return cls

            # ================= attention (j-pipelined) =================
            for j in range(2):
                if j == 0:
                    # trickle V heads 8-15 into the first half of j0
                    nc.sync.dma_start(out=bv_b, in_=bcast(bvh[:], D)[:, NW:D])
                    fillers = [(lambda sch=sch: v_group(sch, 1))
                               for sch in range(SCH)]
                else:
                    relu_j[0] = mp.tile([P, 32, NW], bf16, tag="slotA",
                                        name="relu_j0")
                    fillers = offn_closures(0)
                fi = [0]

                def run_filler():
                    if fi[0] < len(fillers):
                        fillers[fi[0]]()
                        fi[0] += 1

                with nc.named_scope(f"attn{j}"):
                    for hp in range(DCH):
                        pats = [patp.tile([P, 4, DK + 1], f32,
                                          tag=f"pat{hs}", name=f"pat{hs}")
                                for hs in range(2)]
                        ets = {}

                        def sc_exp(g, hp=hp, j=j, ets=ets):
                            st = scp.tile([P, 2, NW], f32, tag="sc")
                            for hs in range(2):
                                p0 = hs * 64
                                nc.tensor.matmul(
                                    st[:, hs, :],
                                    ktb[p0:p0 + 64, hp, g * P:(g + 1) * P],
                                    qtb[p0:p0 + 64, 2 * hp + hs,
                                        j * NW:(j + 1) * NW],
                                    start=True, stop=True)
                            et = etp.tile([P, 2, NW], bf16, tag="et")
                            nc.scalar.activation(
                                et, st, AF.Exp, scale=float(1.0 / np.sqrt(DK)))
                            ets[g] = et

                        def a_v(g, hp=hp, pats=pats, ets=ets):
                            et = ets.pop(g)
                            for hs in range(2):
                                h = 2 * hp + hs
                                for q in range(4):
                                    nc.tensor.matmul(
                                        pats[hs][:, q, :],
                                        et[:, hs, q * P:(q + 1) * P],
                                        vaug[:, g, h * (DK + 1):
                                             (h + 1) * (DK + 1)],
                                        start=(g == 0 and q == 0),
                                        stop=(g == SCH - 1 and q == 3))

                        for gg in range(8):
                            if j == 1 or gg % 2 == 0:
                                run_filler()
                            if j == 1 and hp == 0:
                                run_filler()  # front-load O-j0 into pair 0
                            sc_exp(2 * gg)
                            sc_exp(2 * gg + 1)
                            if gg >= 1:
                                a_v(2 * gg - 2)
                                a_v(2 * gg - 1)
                        a_v(14)
                        a_v(15)

                        # normalize + assemble + transpose
                        at2 = [at2p.tile([P, P], bf16, tag="at2",
                                         name=f"at2_{q}") for q in range(4)]
                        for hs in range(2):
                            rec4 = smp.tile([P, 4, 1], f32, tag="rec4")
                            nc.vector.reciprocal(
                                rec4, pats[hs][:, :, DK:DK + 1])
                            for q in range(4):
                                nc.vector.tensor_scalar_mul(
                                    at2[q][:, hs * DK:(hs + 1) * DK],
                                    pats[hs][:, q, 0:DK], rec4[:, q, :])
                        for q in range(4):
                            ptr = trp.tile([P, P], bf16, tag="tr")
                            nc.tensor.transpose(ptr, at2[q], ident_bf)
                            nc.vector.tensor_copy(
                                concat[j][:, hp, q * P:(q + 1) * P], ptr)

                # drain leftover fillers for this window
                with nc.named_scope(f"drain{j}"):
                    while fi[0] < len(fillers):
                        run_filler()

            # ================= tail: O/LN/FFN for j1 =================
            relu_j[1] = mp.tile([P, 32, NW], bf16, tag="slotC",
                                name="relu_j1")
            misc_state["tail"] = True
            with nc.named_scope("tail"):
                for c in offn_closures(1):
                    c()

    nc.compile()
    return nc


def _get_program():
    global _PROG
    if _PROG is None:
        _PROG = _build_program()
    return _PROG


def make_in_maps(x, Wq, bq, Wk, bk, Wv, bv, Wo, bo, alpha1, bias1, alpha2,
                 bias2, W1, b1, W2, b2):
    """Build the 8 per-core input maps. Shared arrays are reused by reference."""
    def b16(a):
        return np.ascontiguousarray(a).astype(BF16)

    shared = {
        "wq": b16(Wq), "wk": b16(Wk), "wv": b16(Wv), "wo": b16(Wo),
        "w1": b16(np.asarray(alpha1, np.float32)[:, None] * np.asarray(W1, np.float32)),
        "w2": b16(W2),
        "bq": np.asarray(bq, np.float32), "bk": np.asarray(bk, np.float32),
        "bvh": b16(bv),
        "b1": (np.asarray(b1, np.float32)
               + np.asarray(bias1, np.float32) @ np.asarray(W1, np.float32)),
        "alpha1": b16(alpha1),
        "beta1": b16(np.asarray(bias1, np.float32) + np.asarray(b2, np.float32)),
        "alpha2": b16(alpha2),
        "beta2": b16(bias2),
    }
    x = np.asarray(x, np.float32)
    bo = np.asarray(bo, np.float32)
    in_maps = []
    for c in range(NCORES):
        b, j = c // 2, c % 2
        xb = x[b]
        if j == 0:
            xt_np = xb.T
        else:
            xt_np = np.concatenate([xb[SQ:].T, xb[:SQ].T], axis=1)
        m = dict(shared)
        m["xt"] = b16(xt_np)
        m["xh"] = b16(xb[j * SQ:(j + 1) * SQ] + bo[None, :])
        in_maps.append(m)
    return in_maps


def kernel(**inputs):
    from concourse.bass_utils import run_bass_kernel_spmd

    nc = _get_program()
    in_maps = make_in_maps(**inputs)
    res = run_bass_kernel_spmd(nc, in_maps, core_ids=list(range(NCORES)))
    out = np.empty((B, S, D), np.float32)
    for c in range(NCORES):
        b, j = c // 2, c % 2
        out[b, j * SQ:(j + 1) * SQ, :] = res.results[c]["out"]
    return out


# revision 24
# speedup vs baseline: 1.0447x; 1.0447x over previous
"""Transformer encoder layer (post-norm, 16 heads, d_model=1024, d_ff=4096)
on 8 Trainium2 NeuronCores.

Sharding: batch(4) x seq-half(2) -> 8 shards. Each core computes K/V for its
batch's FULL sequence and Q/attention/FFN/LN for its 1024-query half.

v2: software-pipelined. The softmax exp (ACT engine) is the attention
bottleneck; the kernel interleaves the second half of the V-projection into
query-block j0's attention and all of O-proj/LN1/FFN for j0 into j1's
attention, so the PE has dense work while ACT churns through exp. Scores
use 64x128 row-tiled matmuls (two heads run concurrently in the PE array,
K=64 each, no zero padding); exp runs on 2-bank PSUM groups ([P,2,512] per
instruction) to amortize ACT access overhead. attn@V accumulates 8
independent [128,65] regions across two single-bank PSUM tiles (one
start/stop pair per bank). All PE transposes are bf16. Softmax skips
max-subtraction (scores ~ N(0,1)) - identical after normalization.
"""

import numpy as np
import ml_dtypes

B, S, D = 4, 2048, 1024
H, DK = 16, 64
DFF = 4096
SQ = S // 2          # queries per core
P = 128              # partitions
EPS = 1e-6
NCORES = 8

BF16 = ml_dtypes.bfloat16

_PROG = None  # cached compiled program


def _build_program():
    import concourse.bacc as bacc
    import concourse.tile as tile
    import concourse.mybir as mybir
    from concourse.masks import make_identity

    f32 = mybir.dt.float32
    bf16 = mybir.dt.bfloat16
    AF = mybir.ActivationFunctionType
    Alu = mybir.AluOpType

    nc = bacc.Bacc("TRN2", target_bir_lowering=False, debug=False,
                   num_devices=NCORES)

    # ---- DRAM parameters (per-core shards supplied by host) ----
    xt = nc.declare_dram_parameter("xt", [D, S], bf16, isOutput=False)    # x[b].T
    xh = nc.declare_dram_parameter("xh", [SQ, D], bf16, isOutput=False)   # x_half + bo
    wq = nc.declare_dram_parameter("wq", [D, D], bf16, isOutput=False)
    wk = nc.declare_dram_parameter("wk", [D, D], bf16, isOutput=False)
    wv = nc.declare_dram_parameter("wv", [D, D], bf16, isOutput=False)
    wo = nc.declare_dram_parameter("wo", [D, D], bf16, isOutput=False)
    w1 = nc.declare_dram_parameter("w1", [D, DFF], bf16, isOutput=False)
    w2 = nc.declare_dram_parameter("w2", [DFF, D], bf16, isOutput=False)
    bq = nc.declare_dram_parameter("bq", [D], f32, isOutput=False)
    bk = nc.declare_dram_parameter("bk", [D], f32, isOutput=False)
    bvh = nc.declare_dram_parameter("bvh", [D], bf16, isOutput=False)
    b1p = nc.declare_dram_parameter("b1", [DFF], f32, isOutput=False)
    a1p = nc.declare_dram_parameter("alpha1", [D], bf16, isOutput=False)
    g1p = nc.declare_dram_parameter("beta1", [D], bf16, isOutput=False)
    a2p = nc.declare_dram_parameter("alpha2", [D], bf16, isOutput=False)
    g2p = nc.declare_dram_parameter("beta2", [D], bf16, isOutput=False)
    out = nc.declare_dram_parameter("out", [SQ, D], f32, isOutput=True)

    KC = D // P          # 8 k-chunks of 128
    DCH = D // P         # 8 feature chunks
    SCH = S // P         # 16 s-chunks
    NW = 512

    import concourse.bass as bass

    def bcast(ap_1d, n):
        return bass.AP(tensor=ap_1d.tensor, offset=ap_1d.offset,
                       ap=[[0, P]] + list(ap_1d.ap[-1:]))[:, 0:n]

    with tile.TileContext(nc) as tc:
        with tc.tile_pool(name="main", bufs=1) as mp, \
             tc.tile_pool(name="wp", bufs=2) as wp, \
             tc.tile_pool(name="etp", bufs=4) as etp, \
             tc.tile_pool(name="at2p", bufs=4) as at2p, \
             tc.tile_pool(name="small", bufs=4) as smp, \
             tc.tile_pool(name="tokp", bufs=1) as tokp, \
             tc.tile_pool(name="outp", bufs=1) as outp, \
             tc.tile_pool(name="scp", bufs=2, space="PSUM") as scp, \
             tc.tile_pool(name="patp", bufs=1, space="PSUM") as patp, \
             tc.tile_pool(name="trp", bufs=1, space="PSUM") as trp, \
             tc.tile_pool(name="miscp", bufs=1, space="PSUM") as miscp:

            # ---- constants ----
            ident_bf = mp.tile([P, P], bf16, tag="ident_bf")
            make_identity(nc, ident_bf)

            bq_sb = mp.tile([P, DCH], f32, tag="bq")
            nc.sync.dma_start(out=bq_sb, in_=bq[:].rearrange("(c p) -> p c", p=P))
            bk_sb = mp.tile([P, DCH], f32, tag="bk")
            nc.sync.dma_start(out=bk_sb, in_=bk[:].rearrange("(c p) -> p c", p=P))
            b1_sb = mp.tile([P, DFF // P], f32, tag="b1")
            nc.sync.dma_start(out=b1_sb, in_=b1p[:].rearrange("(c p) -> p c", p=P))
            bv_b = mp.tile([P, NW], bf16, tag="bv_b")
            nc.sync.dma_start(out=bv_b, in_=bcast(bvh[:], D)[:, 0:NW])
            a1_b = mp.tile([P, D], bf16, tag="a1_b")
            nc.sync.dma_start(out=a1_b, in_=bcast(a1p[:], D))
            g1_b = mp.tile([P, D], bf16, tag="g1_b")
            nc.sync.dma_start(out=g1_b, in_=bcast(g1p[:], D))
            a2_b = mp.tile([P, D], bf16, tag="a2_b")
            nc.sync.dma_start(out=a2_b, in_=bcast(a2p[:], D))
            g2_b = mp.tile([P, D], bf16, tag="g2_b")
            nc.sync.dma_start(out=g2_b, in_=bcast(g2p[:], D))

            # prepay the exp ACT table load
            warm = mp.tile([P, 1], f32, tag="warm")
            nc.vector.memset(warm, 0.0)
            nc.scalar.activation(warm, warm, AF.Exp)

            # ---- persistent SBUF tensors ----
            # slotA: xtb -> relu_j0 ; slotC: qtb -> relu_j1
            xtb = mp.tile([P, KC, S], bf16, tag="slotA")
            for xh_ in range(2):
                nc.sync.dma_start(
                    out=xtb[:, :, xh_ * SQ:(xh_ + 1) * SQ],
                    in_=xt[:, xh_ * SQ:(xh_ + 1) * SQ].rearrange(
                        "(c p) s -> p c s", p=P))

            ktb = mp.tile([P, DCH, S], bf16, tag="slotB")
            qtb = mp.tile([P, H, SQ], bf16, tag="slotC")
            vaug = mp.tile([P, SCH, H * (DK + 1)], bf16, tag="slotD")
            va_view = vaug.rearrange("p s (h w) -> p s h w", w=DK + 1)
            nc.vector.memset(va_view[:, :, :, DK:DK + 1], 1.0)

            # j0's O-proj is fully front-loaded into j1's first attention
            # pair, so concat-j0 is dead before concat-j1's first write:
            # one shared buffer (WAR tracked by Tile).
            cc_t = mp.tile([P, DCH, NW], bf16, tag="cc")
            concat = [cc_t, cc_t]
            norm1r = mp.tile([P, 4, D], bf16, tag="n1r")     # affined residual
            norm1T = mp.tile([P, DCH, NW], bf16, tag="n1T")  # z transposed
            s2t = mp.tile([P, 4, D], bf16, tag="s2t")        # ffn2 + residual

            # weight chunk loader: [P, KC, 512] slices (8 KB each)
            def wload(src, col0):
                t = wp.tile([P, KC, NW], bf16, tag="w")
                nc.sync.dma_start(
                    out=t, in_=src[:, col0:col0 + NW].rearrange(
                        "(c p) n -> p c n", p=P))
                return t

            def w2load(col0):
                # all 32 dff-chunks for a 128-col slice of W2 (8 KB)
                t = wp.tile([P, DFF // P, P], bf16, tag="w")
                nc.sync.dma_start(
                    out=t, in_=w2[:, col0:col0 + P]
                    .rearrange("(c p) n -> p c n", p=P))
                return t

            wv_c = [None, None]
            wo_c = [None, None]
            w1_c = [None] * 8
            w2_c = [None] * 8

            # misc psum provider: in-window -> single miscp bank;
            # tail -> rotate through scp banks (free then) to avoid WAR stalls
            misc_state = {"tail": False, "i": 0, "tile": None}

            def misc_bank():
                if not misc_state["tail"]:
                    return miscp.tile([P, NW], f32, tag="m", name="mb")
                i = misc_state["i"]
                if i % 2 == 0:
                    misc_state["tile"] = scp.tile([P, 2, NW], f32, tag="sc",
                                                  name="mb2")
                misc_state["i"] = i + 1
                return misc_state["tile"][:, i % 2, :]

            # ================= K projection =================
            with nc.named_scope("qkv"):
                wk_c = [wload(wk, 0), wload(wk, NW)]
                wq_c = [wload(wq, 0), wload(wq, NW)]
                for dch in range(DCH):
                    wch = wk_c[dch // 4]
                    wcol = (dch % 4) * P
                    for half in range(2):
                        st = scp.tile([P, 2, NW], f32, tag="sc")
                        for n2 in range(2):
                            for kc in range(KC):
                                nc.tensor.matmul(
                                    st[:, n2, :],
                                    wch[:, kc, wcol:wcol + P],
                                    xtb[:, kc, (half * 2 + n2) * NW:
                                        (half * 2 + n2 + 1) * NW],
                                    start=(kc == 0), stop=(kc == KC - 1))
                        nc.scalar.activation(
                            ktb[:, dch, half * 1024:(half + 1) * 1024]
                            .rearrange("p (a b) -> p a b", a=2),
                            st, AF.Identity, bias=bk_sb[:, dch:dch + 1])

                # ================= Q projection =================
                wv_c[0] = wload(wv, 0)
                wv_c[1] = wload(wv, NW)
                for dch in range(DCH):
                    wch = wq_c[dch // 4]
                    wcol = (dch % 4) * P
                    st = scp.tile([P, 2, NW], f32, tag="sc")
                    for n2 in range(2):
                        for kc in range(KC):
                            nc.tensor.matmul(
                                st[:, n2, :],
                                wch[:, kc, wcol:wcol + P],
                                xtb[:, kc, n2 * NW:(n2 + 1) * NW],
                                start=(kc == 0), stop=(kc == KC - 1))
                    stf = st.rearrange("p a b -> p (a b)")
                    nc.scalar.activation(
                        qtb[0:64, 2 * dch, :], stf[0:64, :],
                        AF.Identity, bias=bq_sb[0:64, dch:dch + 1])
                    nc.scalar.activation(
                        qtb[64:128, 2 * dch + 1, :], stf[64:128, :],
                        AF.Identity, bias=bq_sb[64:128, dch:dch + 1])

                # V projection, heads 0-7 (n=0), all sequence chunks, upfront
                def v_group(sch, n):
                    pt = misc_bank()
                    for kc in range(KC):
                        nc.tensor.matmul(
                            pt, xtb[:, kc, sch * P:(sch + 1) * P],
                            wv_c[n][:, kc, :],
                            start=(kc == 0), stop=(kc == KC - 1))
                    h0 = n * (NW // DK)
                    nc.vector.tensor_add(
                        va_view[:, sch, h0:h0 + 8, 0:DK],
                        pt.rearrange("p (h w) -> p h w", w=DK),
                        bv_b.rearrange("p (h w) -> p h w", w=DK))

                for sch in range(SCH):
                    v_group(sch, 0)

            # ---------- O-proj + LN1 + FFN closures for query-block j ------
            relu_j = [None, None]

            def ln_sqrt_rec(mv, corr):
                std_t = smp.tile([P, 1], f32, tag="std")
                rec_t = smp.tile([P, 1], f32, tag="recs")
                mean_t = smp.tile([P, 1], f32, tag="mean")
                nc.scalar.activation(std_t, mv[:, 1:2], AF.Sqrt,
                                     scale=float(corr))
                nc.vector.tensor_scalar_add(std_t, std_t, float(EPS))
                nc.vector.reciprocal(rec_t, std_t)
                nc.vector.tensor_copy(mean_t, mv[:, 0:1])
                return mean_t, rec_t

            def offn_closures(j):
                """64 closures: O+add(8), LN1(4), FFN1(32), FFN2(16), LN2(4)."""
                cls = []
                n1f = [None]

                def o_group(sq, n):
                    if sq == 0 and n == 0 and not misc_state["tail"]:
                        wo_c[0] = wload(wo, 0)
                        wo_c[1] = wload(wo, NW)
                    pt = misc_bank()
                    for kc in range(KC):
                        nc.tensor.matmul(
                            pt, concat[j][:, kc, sq * P:(sq + 1) * P],
                            wo_c[n][:, kc, :],
                            start=(kc == 0), stop=(kc == KC - 1))
                    if n == 0:
                        xh_t = tokp.tile([P, D], bf16, tag="tok")
                        nc.sync.dma_start(
                            out=xh_t,
                            in_=xh[(j * 4 + sq) * P:(j * 4 + sq + 1) * P, :])
                        n1f[0] = (outp.tile([P, D], f32, tag="of",
                                            name="n1f"), xh_t)
                    s1, xh_t = n1f[0]
                    nc.vector.tensor_add(
                        s1[:, n * NW:(n + 1) * NW], pt,
                        xh_t[:, n * NW:(n + 1) * NW])

                def ln1(sq):
                    # prefetch the first FFN1 weight chunk
                    if sq == 2:
                        w1_c[0] = wload(w1, 0)
                    s1, _ = n1f[0]
                    stats = smp.tile([P, 2, 6], f32, tag="stats")
                    nc.vector.bn_stats(stats[:, 0, :], s1[:, 0:NW])
                    nc.vector.bn_stats(stats[:, 1, :], s1[:, NW:D])
                    mv = smp.tile([P, 2], f32, tag="mv")
                    nc.vector.bn_aggr(mv, stats)
                    mean_t, rec_t = ln_sqrt_rec(mv, D / (D - 1))
                    zb = smp.tile([P, D], bf16, tag="zb", bufs=1)
                    nc.vector.tensor_scalar(
                        zb, s1, mean_t, rec_t, op0=Alu.subtract, op1=Alu.mult)
                    # affined residual for FFN2 (beta1 has b2 folded host-side)
                    nc.gpsimd.tensor_mul(norm1r[:, sq, :], zb, a1_b)
                    nc.gpsimd.tensor_add(norm1r[:, sq, :], norm1r[:, sq, :],
                                         g1_b)
                    # transpose plain z -> norm1T (alpha1 folded into W1)
                    for dch in range(DCH):
                        ptr = trp.tile([P, P], bf16, tag="tr")
                        nc.tensor.transpose(
                            ptr, zb[:, dch * P:(dch + 1) * P], ident_bf)
                        nc.vector.tensor_copy(
                            norm1T[:, dch, sq * P:(sq + 1) * P], ptr)

                def ffn1(t):
                    # prefetch next w1 chunk / first w2 chunks one use ahead
                    if t % 4 == 0 and t // 4 + 1 < 8:
                        w1_c[t // 4 + 1] = wload(w1, (t // 4 + 1) * 4 * P)
                    if t == 28:
                        w2_c[0] = w2load(0)
                    if t == 29:
                        w2_c[1] = w2load(P)
                    wch = w1_c[t // 4]
                    pt = misc_bank()
                    for kc in range(KC):
                        nc.tensor.matmul(
                            pt, wch[:, kc, (t % 4) * P:(t % 4 + 1) * P],
                            norm1T[:, kc, :],
                            start=(kc == 0), stop=(kc == KC - 1))
                    nc.scalar.activation(relu_j[j][:, t, :], pt, AF.Relu,
                                         bias=b1_sb[:, t:t + 1])

                def ffn2(c, sq):
                    # c indexes 128-col slices of the output; prefetch c+2
                    if sq == 0 and c + 2 < 8:
                        w2_c[c + 2] = w2load((c + 2) * P)
                    pt = misc_bank()[:, 0:P]
                    for kc in range(DFF // P):
                        nc.tensor.matmul(
                            pt, relu_j[j][:, kc, sq * P:(sq + 1) * P],
                            w2_c[c][:, kc, :],
                            start=(kc == 0), stop=(kc == DFF // P - 1))
                    nc.vector.tensor_add(
                        s2t[:, sq, c * P:(c + 1) * P], pt,
                        norm1r[:, sq, c * P:(c + 1) * P])

                def ln2(sq):
                    s2 = s2t[:, sq, :]
                    stats = smp.tile([P, 2, 6], f32, tag="stats")
                    nc.vector.bn_stats(stats[:, 0, :], s2[:, 0:NW])
                    nc.vector.bn_stats(stats[:, 1, :], s2[:, NW:D])
                    mv = smp.tile([P, 2], f32, tag="mv")
                    nc.vector.bn_aggr(mv, stats)
                    mean_t, rec_t = ln_sqrt_rec(mv, D / (D - 1))
                    of = outp.tile([P, D], f32, tag="of")
                    nc.vector.tensor_scalar(
                        of, s2, mean_t, rec_t, op0=Alu.subtract, op1=Alu.mult)
                    nc.gpsimd.tensor_mul(of[:, 0:NW], of[:, 0:NW],
                                         a2_b[:, 0:NW])
                    nc.vector.tensor_mul(of[:, NW:D], of[:, NW:D],
                                         a2_b[:, NW:D])
                    nc.gpsimd.tensor_add(of[:, 0:NW], of[:, 0:NW],
                                         g2_b[:, 0:NW])
                    nc.vector.tensor_add(of[:, NW:D], of[:, NW:D],
                                         g2_b[:, NW:D])
                    r0 = (j * 4 + sq) * P
                    nc.sync.dma_start(out=out[r0:r0 + P, :], in_=of)

                for sq in range(4):
                    cls.append(lambda sq=sq: o_group(sq, 0))
                    cls.append(lambda sq=sq: o_group(sq, 1))
                    cls.append(lambda sq=sq: ln1(sq))
                for t in range(32):
                    cls.append(lambda t=t: ffn1(t))
                for c in range(8):
                    for sq in range(4):
                        cls.append(lambda c=c, sq=sq: ffn2(c, sq))
                for sq in range(4):
                    cls.append(lambda sq=sq: ln2(sq))
                return cls

            # ================= attention (j-pipelined) =================
            for j in range(2):
                if j == 0:
                    # trickle V heads 8-15 into the first half of j0
                    nc.sync.dma_start(out=bv_b, in_=bcast(bvh[:], D)[:, NW:D])
                    fillers = [(lambda sch=sch: v_group(sch, 1))
                               for sch in range(SCH)]
                else:
                    relu_j[0] = mp.tile([P, 32, NW], bf16, tag="slotA",
                                        name="relu_j0")
                    fillers = offn_closures(0)
                fi = [0]

                def run_filler():
                    if fi[0] < len(fillers):
                        fillers[fi[0]]()
                        fi[0] += 1

                with nc.named_scope(f"attn{j}"):
                    for hp in range(DCH):
                        pats = [patp.tile([P, 4, DK + 1], f32,
                                          tag=f"pat{hs}", name=f"pat{hs}")
                                for hs in range(2)]
                        ets = {}

                        def sc_exp(g, hp=hp, j=j, ets=ets):
                            st = scp.tile([P, 2, NW], f32, tag="sc")
                            for hs in range(2):
                                p0 = hs * 64
                                nc.tensor.matmul(
                                    st[:, hs, :],
                                    ktb[p0:p0 + 64, hp, g * P:(g + 1) * P],
                                    qtb[p0:p0 + 64, 2 * hp + hs,
                                        j * NW:(j + 1) * NW],
                                    start=True, stop=True)
                            et = etp.tile([P, 2, NW], bf16, tag="et")
                            nc.scalar.activation(
                                et, st, AF.Exp, scale=float(1.0 / np.sqrt(DK)))
                            ets[g] = et

                        def a_v(g, hp=hp, pats=pats, ets=ets):
                            et = ets.pop(g)
                            for hs in range(2):
                                h = 2 * hp + hs
                                for q in range(4):
                                    nc.tensor.matmul(
                                        pats[hs][:, q, :],
                                        et[:, hs, q * P:(q + 1) * P],
                                        vaug[:, g, h * (DK + 1):
                                             (h + 1) * (DK + 1)],
                                        start=(g == 0 and q == 0),
                                        stop=(g == SCH - 1 and q == 3))

                        for gg in range(8):
                            if j == 1 or gg % 2 == 0:
                                run_filler()
                            if j == 1 and hp == 0:
                                run_filler()  # front-load O-j0 into pair 0
                            sc_exp(2 * gg)
                            sc_exp(2 * gg + 1)
                            if gg >= 1:
                                a_v(2 * gg - 2)
                                a_v(2 * gg - 1)
                        a_v(14)
                        a_v(15)

                        # normalize + assemble + transpose
                        at2 = [at2p.tile([P, P], bf16, tag="at2",
                                         name=f"at2_{q}") for q in range(4)]
                        for hs in range(2):
                            rec4 = smp.tile([P, 4, 1], f32, tag="rec4")
                            nc.vector.reciprocal(
                                rec4, pats[hs][:, :, DK:DK + 1])
                            for q in range(4):
                                nc.vector.tensor_scalar_mul(
                                    at2[q][:, hs * DK:(hs + 1) * DK],
                                    pats[hs][:, q, 0:DK], rec4[:, q, :])
                        for q in range(4):
                            ptr = trp.tile([P, P], bf16, tag="tr")
                            nc.tensor.transpose(ptr, at2[q], ident_bf)
                            nc.vector.tensor_copy(
                                concat[j][:, hp, q * P:(q + 1) * P], ptr)

                # drain leftover fillers for this window
                with nc.named_scope(f"drain{j}"):
                    while fi[0] < len(fillers):
                        run_filler()

            # ================= tail: O/LN/FFN for j1 =================
            relu_j[1] = mp.tile([P, 32, NW], bf16, tag="slotC",
                                name="relu_j1")
            misc_state["tail"] = True
            # preload tail's O weights while j1's last assembly drains
            wo_c[0] = wload(wo, 0)
            wo_c[1] = wload(wo, NW)
            with nc.named_scope("tail"):
                for c in offn_closures(1):
                    c()

    nc.compile()
    return nc


def _get_program():
    global _PROG
    if _PROG is None:
        _PROG = _build_program()
    return _PROG


def make_in_maps(x, Wq, bq, Wk, bk, Wv, bv, Wo, bo, alpha1, bias1, alpha2,
                 bias2, W1, b1, W2, b2):
    """Build the 8 per-core input maps. Shared arrays are reused by reference."""
    def b16(a):
        return np.ascontiguousarray(a).astype(BF16)

    shared = {
        "wq": b16(Wq), "wk": b16(Wk), "wv": b16(Wv), "wo": b16(Wo),
        "w1": b16(np.asarray(alpha1, np.float32)[:, None] * np.asarray(W1, np.float32)),
        "w2": b16(W2),
        "bq": np.asarray(bq, np.float32), "bk": np.asarray(bk, np.float32),
        "bvh": b16(bv),
        "b1": (np.asarray(b1, np.float32)
               + np.asarray(bias1, np.float32) @ np.asarray(W1, np.float32)),
        "alpha1": b16(alpha1),
        "beta1": b16(np.asarray(bias1, np.float32) + np.asarray(b2, np.float32)),
        "alpha2": b16(alpha2),
        "beta2": b16(bias2),
    }
    x = np.asarray(x, np.float32)
    bo = np.asarray(bo, np.float32)
    in_maps = []
    for c in range(NCORES):
        b, j = c // 2, c % 2
        xb = x[b]
        if j == 0:
            xt_np = xb.T
        else:
            xt_np = np.concatenate([xb[SQ:].T, xb[:SQ].T], axis=1)
        m = dict(shared)
        m["xt"] = b16(xt_np)
        m["xh"] = b16(xb[j * SQ:(j + 1) * SQ] + bo[None, :])
        in_maps.append(m)
    return in_maps


def kernel(**inputs):
    from concourse.bass_utils import run_bass_kernel_spmd

    nc = _get_program()
    in_maps = make_in_maps(**inputs)
    res = run_bass_kernel_spmd(nc, in_maps, core_ids=list(range(NCORES)))
    out = np.empty((B, S, D), np.float32)
    for c in range(NCORES):
        b, j = c // 2, c % 2
        out[b, j * SQ:(j + 1) * SQ, :] = res.results[c]["out"]
    return out


# revision 27
# speedup vs baseline: 1.0527x; 1.0077x over previous
"""Transformer encoder layer (post-norm, 16 heads, d_model=1024, d_ff=4096)
on 8 Trainium2 NeuronCores.

Sharding: batch(4) x seq-half(2) -> 8 shards. Each core computes K/V for its
batch's FULL sequence and Q/attention/FFN/LN for its 1024-query half.

v2: software-pipelined. The softmax exp (ACT engine) is the attention
bottleneck; the kernel interleaves the second half of the V-projection into
query-block j0's attention and all of O-proj/LN1/FFN for j0 into j1's
attention, so the PE has dense work while ACT churns through exp. Scores
use 64x128 row-tiled matmuls (two heads run concurrently in the PE array,
K=64 each, no zero padding); exp runs on 2-bank PSUM groups ([P,2,512] per
instruction) to amortize ACT access overhead. attn@V accumulates 8
independent [128,65] regions across two single-bank PSUM tiles (one
start/stop pair per bank). All PE transposes are bf16. Softmax skips
max-subtraction (scores ~ N(0,1)) - identical after normalization.
"""

import numpy as np
import ml_dtypes

B, S, D = 4, 2048, 1024
H, DK = 16, 64
DFF = 4096
SQ = S // 2          # queries per core
P = 128              # partitions
EPS = 1e-6
NCORES = 8

BF16 = ml_dtypes.bfloat16

_PROG = None  # cached compiled program


def _build_program():
    import concourse.bacc as bacc
    import concourse.tile as tile
    import concourse.mybir as mybir
    from concourse.masks import make_identity

    f32 = mybir.dt.float32
    bf16 = mybir.dt.bfloat16
    AF = mybir.ActivationFunctionType
    Alu = mybir.AluOpType

    nc = bacc.Bacc("TRN2", target_bir_lowering=False, debug=False,
                   num_devices=NCORES)

    # ---- DRAM parameters (per-core shards supplied by host) ----
    xt = nc.declare_dram_parameter("xt", [D, S], bf16, isOutput=False)    # x[b].T
    xh = nc.declare_dram_parameter("xh", [SQ, D], bf16, isOutput=False)   # x_half + bo
    wq = nc.declare_dram_parameter("wq", [D, D], bf16, isOutput=False)
    wk = nc.declare_dram_parameter("wk", [D, D], bf16, isOutput=False)
    wv = nc.declare_dram_parameter("wv", [D, D], bf16, isOutput=False)
    wo = nc.declare_dram_parameter("wo", [D, D], bf16, isOutput=False)
    w1 = nc.declare_dram_parameter("w1", [D, DFF], bf16, isOutput=False)
    w2 = nc.declare_dram_parameter("w2", [DFF, D], bf16, isOutput=False)
    bq = nc.declare_dram_parameter("bq", [D], f32, isOutput=False)
    bk = nc.declare_dram_parameter("bk", [D], f32, isOutput=False)
    bvh = nc.declare_dram_parameter("bvh", [D], bf16, isOutput=False)
    b1p = nc.declare_dram_parameter("b1", [DFF], f32, isOutput=False)
    a1p = nc.declare_dram_parameter("alpha1", [D], bf16, isOutput=False)
    g1p = nc.declare_dram_parameter("beta1", [D], bf16, isOutput=False)
    a2p = nc.declare_dram_parameter("alpha2", [D], bf16, isOutput=False)
    g2p = nc.declare_dram_parameter("beta2", [D], bf16, isOutput=False)
    out = nc.declare_dram_parameter("out", [SQ, D], f32, isOutput=True)

    KC = D // P          # 8 k-chunks of 128
    DCH = D // P         # 8 feature chunks
    SCH = S // P         # 16 s-chunks
    NW = 512

    import concourse.bass as bass

    def bcast(ap_1d, n):
        return bass.AP(tensor=ap_1d.tensor, offset=ap_1d.offset,
                       ap=[[0, P]] + list(ap_1d.ap[-1:]))[:, 0:n]

    with tile.TileContext(nc) as tc:
        with tc.tile_pool(name="main", bufs=1) as mp, \
             tc.tile_pool(name="wp", bufs=2) as wp, \
             tc.tile_pool(name="etp", bufs=4) as etp, \
             tc.tile_pool(name="at2p", bufs=4) as at2p, \
             tc.tile_pool(name="small", bufs=4) as smp, \
             tc.tile_pool(name="tokp", bufs=1) as tokp, \
             tc.tile_pool(name="outp", bufs=1) as outp, \
             tc.tile_pool(name="scp", bufs=2, space="PSUM") as scp, \
             tc.tile_pool(name="patp", bufs=1, space="PSUM") as patp, \
             tc.tile_pool(name="trp", bufs=1, space="PSUM") as trp, \
             tc.tile_pool(name="miscp", bufs=1, space="PSUM") as miscp:

            # ---- constants ----
            ident_bf = mp.tile([P, P], bf16, tag="ident_bf")
            make_identity(nc, ident_bf)

            bq_sb = mp.tile([P, DCH], f32, tag="bq")
            nc.sync.dma_start(out=bq_sb, in_=bq[:].rearrange("(c p) -> p c", p=P))
            bk_sb = mp.tile([P, DCH], f32, tag="bk")
            nc.sync.dma_start(out=bk_sb, in_=bk[:].rearrange("(c p) -> p c", p=P))
            b1_sb = mp.tile([P, DFF // P], f32, tag="b1")
            nc.sync.dma_start(out=b1_sb, in_=b1p[:].rearrange("(c p) -> p c", p=P))
            bv_b = mp.tile([P, NW], bf16, tag="bv_b")
            nc.sync.dma_start(out=bv_b, in_=bcast(bvh[:], D)[:, 0:NW])
            a1_b = mp.tile([P, D], bf16, tag="a1_b")
            nc.sync.dma_start(out=a1_b, in_=bcast(a1p[:], D))
            g1_b = mp.tile([P, D], bf16, tag="g1_b")
            nc.sync.dma_start(out=g1_b, in_=bcast(g1p[:], D))
            a2_b = mp.tile([P, D], bf16, tag="a2_b")
            nc.sync.dma_start(out=a2_b, in_=bcast(a2p[:], D))
            g2_b = mp.tile([P, D], bf16, tag="g2_b")
            nc.sync.dma_start(out=g2_b, in_=bcast(g2p[:], D))

            # prepay the exp ACT table load
            warm = mp.tile([P, 1], f32, tag="warm")
            nc.vector.memset(warm, 0.0)
            nc.scalar.activation(warm, warm, AF.Exp)

            # ---- persistent SBUF tensors ----
            # slotA: xtb -> relu_j0 ; slotC: qtb -> relu_j1
            xtb = mp.tile([P, KC, S], bf16, tag="slotA")
            for xh_ in range(2):
                nc.sync.dma_start(
                    out=xtb[:, :, xh_ * SQ:(xh_ + 1) * SQ],
                    in_=xt[:, xh_ * SQ:(xh_ + 1) * SQ].rearrange(
                        "(c p) s -> p c s", p=P))

            ktb = mp.tile([P, DCH, S], bf16, tag="slotB")
            qtb = mp.tile([P, H, SQ], bf16, tag="slotC")
            vaug = mp.tile([P, SCH, H * (DK + 1)], bf16, tag="slotD")
            va_view = vaug.rearrange("p s (h w) -> p s h w", w=DK + 1)
            nc.vector.memset(va_view[:, :, :, DK:DK + 1], 1.0)

            # j0's O-proj is fully front-loaded into j1's first attention
            # pair, so concat-j0 is dead before concat-j1's first write:
            # one shared buffer (WAR tracked by Tile).
            cc_t = mp.tile([P, DCH, NW], bf16, tag="cc")
            concat = [cc_t, cc_t]
            norm1r = mp.tile([P, 4, D], bf16, tag="n1r")     # affined residual
            norm1T = mp.tile([P, DCH, NW], bf16, tag="n1T")  # z transposed
            s2t = mp.tile([P, 4, D], bf16, tag="s2t")        # ffn2 + residual

            # weight chunk loader: [P, KC, 512] slices (8 KB each)
            def wload(src, col0):
                t = wp.tile([P, KC, NW], bf16, tag="w")
                nc.sync.dma_start(
                    out=t, in_=src[:, col0:col0 + NW].rearrange(
                        "(c p) n -> p c n", p=P))
                return t

            def w2load(col0):
                # all 32 dff-chunks for a 128-col slice of W2 (8 KB)
                t = wp.tile([P, DFF // P, P], bf16, tag="w")
                nc.sync.dma_start(
                    out=t, in_=w2[:, col0:col0 + P]
                    .rearrange("(c p) n -> p c n", p=P))
                return t

            wv_c = [None, None]
            wo_c = [None, None]
            w1_c = [None] * 8
            w2_c = [None] * 8

            # misc psum provider: in-window -> single miscp bank;
            # tail -> rotate through scp banks (free then) to avoid WAR stalls
            misc_state = {"tail": False, "i": 0, "tile": None}

            def misc_bank():
                if not misc_state["tail"]:
                    return miscp.tile([P, NW], f32, tag="m", name="mb")
                i = misc_state["i"]
                if i % 2 == 0:
                    misc_state["tile"] = scp.tile([P, 2, NW], f32, tag="sc",
                                                  name="mb2")
                misc_state["i"] = i + 1
                return misc_state["tile"][:, i % 2, :]

            # ================= K projection =================
            with nc.named_scope("qkv"):
                wk_c = [wload(wk, 0), wload(wk, NW)]
                wq_c = [wload(wq, 0), wload(wq, NW)]
                for dch in range(DCH):
                    wch = wk_c[dch // 4]
                    wcol = (dch % 4) * P
                    for half in range(2):
                        st = scp.tile([P, 2, NW], f32, tag="sc")
                        for n2 in range(2):
                            for kc in range(KC):
                                nc.tensor.matmul(
                                    st[:, n2, :],
                                    wch[:, kc, wcol:wcol + P],
                                    xtb[:, kc, (half * 2 + n2) * NW:
                                        (half * 2 + n2 + 1) * NW],
                                    start=(kc == 0), stop=(kc == KC - 1))
                        nc.scalar.activation(
                            ktb[:, dch, half * 1024:(half + 1) * 1024]
                            .rearrange("p (a b) -> p a b", a=2),
                            st, AF.Identity, bias=bk_sb[:, dch:dch + 1])

                # ================= Q projection =================
                wv_c[0] = wload(wv, 0)
                wv_c[1] = wload(wv, NW)
                for dch in range(DCH):
                    wch = wq_c[dch // 4]
                    wcol = (dch % 4) * P
                    st = scp.tile([P, 2, NW], f32, tag="sc")
                    for n2 in range(2):
                        for kc in range(KC):
                            nc.tensor.matmul(
                                st[:, n2, :],
                                wch[:, kc, wcol:wcol + P],
                                xtb[:, kc, n2 * NW:(n2 + 1) * NW],
                                start=(kc == 0), stop=(kc == KC - 1))
                    stf = st.rearrange("p a b -> p (a b)")
                    nc.scalar.activation(
                        qtb[0:64, 2 * dch, :], stf[0:64, :],
                        AF.Identity, bias=bq_sb[0:64, dch:dch + 1])
                    nc.scalar.activation(
                        qtb[64:128, 2 * dch + 1, :], stf[64:128, :],
                        AF.Identity, bias=bq_sb[64:128, dch:dch + 1])

                # V projection, heads 0-7 (n=0), all sequence chunks, upfront
                def v_group(sch, n):
                    pt = misc_bank()
                    for kc in range(KC):
                        nc.tensor.matmul(
                            pt, xtb[:, kc, sch * P:(sch + 1) * P],
                            wv_c[n][:, kc, :],
                            start=(kc == 0), stop=(kc == KC - 1))
                    h0 = n * (NW // DK)
                    nc.vector.tensor_add(
                        va_view[:, sch, h0:h0 + 8, 0:DK],
                        pt.rearrange("p (h w) -> p h w", w=DK),
                        bv_b.rearrange("p (h w) -> p h w", w=DK))

                for sch in range(SCH):
                    v_group(sch, 0)

            # ---------- O-proj + LN1 + FFN closures for query-block j ------
            relu_j = [None, None]

            def ln_sqrt_rec(mv, corr):
                std_t = smp.tile([P, 1], f32, tag="std")
                rec_t = smp.tile([P, 1], f32, tag="recs")
                mean_t = smp.tile([P, 1], f32, tag="mean")
                nc.scalar.activation(std_t, mv[:, 1:2], AF.Sqrt,
                                     scale=float(corr))
                nc.vector.tensor_scalar_add(std_t, std_t, float(EPS))
                nc.vector.reciprocal(rec_t, std_t)
                nc.vector.tensor_copy(mean_t, mv[:, 0:1])
                return mean_t, rec_t

            def offn_closures(j):
                """64 closures: O+add(8), LN1(4), FFN1(32), FFN2(16), LN2(4)."""
                cls = []
                n1f = [None]

                def o_group(sq, n):
                    if sq == 0 and n == 0 and not misc_state["tail"]:
                        wo_c[0] = wload(wo, 0)
                        wo_c[1] = wload(wo, NW)
                    pt = misc_bank()
                    for kc in range(KC):
                        nc.tensor.matmul(
                            pt, concat[j][:, kc, sq * P:(sq + 1) * P],
                            wo_c[n][:, kc, :],
                            start=(kc == 0), stop=(kc == KC - 1))
                    if n == 0:
                        xh_t = tokp.tile([P, D], bf16, tag="tok")
                        nc.sync.dma_start(
                            out=xh_t,
                            in_=xh[(j * 4 + sq) * P:(j * 4 + sq + 1) * P, :])
                        n1f[0] = (outp.tile([P, D], f32, tag="of",
                                            name="n1f"), xh_t)
                    s1, xh_t = n1f[0]
                    nc.vector.tensor_add(
                        s1[:, n * NW:(n + 1) * NW], pt,
                        xh_t[:, n * NW:(n + 1) * NW])

                def ln1(sq):
                    # prefetch the first FFN1 weight chunk
                    if sq == 2:
                        w1_c[0] = wload(w1, 0)
                    s1, _ = n1f[0]
                    stats = smp.tile([P, 2, 6], f32, tag="stats")
                    nc.vector.bn_stats(stats[:, 0, :], s1[:, 0:NW])
                    nc.vector.bn_stats(stats[:, 1, :], s1[:, NW:D])
                    mv = smp.tile([P, 2], f32, tag="mv")
                    nc.vector.bn_aggr(mv, stats)
                    mean_t, rec_t = ln_sqrt_rec(mv, D / (D - 1))
                    zb = smp.tile([P, D], bf16, tag="zb", bufs=1)
                    nc.vector.tensor_scalar(
                        zb, s1, mean_t, rec_t, op0=Alu.subtract, op1=Alu.mult)
                    # affined residual for FFN2 (beta1 has b2 folded host-side)
                    nc.gpsimd.tensor_mul(norm1r[:, sq, :], zb, a1_b)
                    nc.gpsimd.tensor_add(norm1r[:, sq, :], norm1r[:, sq, :],
                                         g1_b)
                    # transpose plain z -> norm1T (alpha1 folded into W1)
                    for dch in range(DCH):
                        ptr = trp.tile([P, P], bf16, tag="tr")
                        nc.tensor.transpose(
                            ptr, zb[:, dch * P:(dch + 1) * P], ident_bf)
                        nc.vector.tensor_copy(
                            norm1T[:, dch, sq * P:(sq + 1) * P], ptr)

                def ffn1(t):
                    # prefetch next w1 chunk / first w2 chunks one use ahead
                    if t % 4 == 0 and t // 4 + 1 < 8:
                        w1_c[t // 4 + 1] = wload(w1, (t // 4 + 1) * 4 * P)
                    if t == 28:
                        w2_c[0] = w2load(0)
                    if t == 29:
                        w2_c[1] = w2load(P)
                    wch = w1_c[t // 4]
                    pt = misc_bank()
                    for kc in range(KC):
                        nc.tensor.matmul(
                            pt, wch[:, kc, (t % 4) * P:(t % 4 + 1) * P],
                            norm1T[:, kc, :],
                            start=(kc == 0), stop=(kc == KC - 1))
                    nc.scalar.activation(relu_j[j][:, t, :], pt, AF.Relu,
                                         bias=b1_sb[:, t:t + 1])

                def ffn2(c, sq):
                    # c indexes 128-col slices of the output; prefetch c+2
                    if sq == 0 and c + 2 < 8:
                        w2_c[c + 2] = w2load((c + 2) * P)
                    pt = misc_bank()[:, 0:P]
                    for kc in range(DFF // P):
                        nc.tensor.matmul(
                            pt, relu_j[j][:, kc, sq * P:(sq + 1) * P],
                            w2_c[c][:, kc, :],
                            start=(kc == 0), stop=(kc == DFF // P - 1))
                    nc.vector.tensor_add(
                        s2t[:, sq, c * P:(c + 1) * P], pt,
                        norm1r[:, sq, c * P:(c + 1) * P])

                def ln2(sq):
                    s2 = s2t[:, sq, :]
                    stats = smp.tile([P, 2, 6], f32, tag="stats")
                    nc.vector.bn_stats(stats[:, 0, :], s2[:, 0:NW])
                    nc.vector.bn_stats(stats[:, 1, :], s2[:, NW:D])
                    mv = smp.tile([P, 2], f32, tag="mv")
                    nc.vector.bn_aggr(mv, stats)
                    mean_t, rec_t = ln_sqrt_rec(mv, D / (D - 1))
                    of = outp.tile([P, D], f32, tag="of")
                    nc.vector.tensor_scalar(
                        of, s2, mean_t, rec_t, op0=Alu.subtract, op1=Alu.mult)
                    nc.gpsimd.tensor_mul(of[:, 0:NW], of[:, 0:NW],
                                         a2_b[:, 0:NW])
                    nc.vector.tensor_mul(of[:, NW:D], of[:, NW:D],
                                         a2_b[:, NW:D])
                    nc.gpsimd.tensor_add(of[:, 0:NW], of[:, 0:NW],
                                         g2_b[:, 0:NW])
                    nc.vector.tensor_add(of[:, NW:D], of[:, NW:D],
                                         g2_b[:, NW:D])
                    r0 = (j * 4 + sq) * P
                    nc.sync.dma_start(out=out[r0:r0 + P, :], in_=of)

                for sq in range(4):
                    cls.append(lambda sq=sq: o_group(sq, 0))
                    cls.append(lambda sq=sq: o_group(sq, 1))
                    cls.append(lambda sq=sq: ln1(sq))
                for t in range(32):
                    cls.append(lambda t=t: ffn1(t))
                for c in range(8):
                    for sq in range(4):
                        cls.append(lambda c=c, sq=sq: ffn2(c, sq))
                for sq in range(4):
                    cls.append(lambda sq=sq: ln2(sq))
                return cls

            # ================= attention (j-pipelined) =================
            for j in range(2):
                if j == 0:
                    # trickle V heads 8-15 into the first half of j0
                    nc.sync.dma_start(out=bv_b, in_=bcast(bvh[:], D)[:, NW:D])
                    fillers = [(lambda sch=sch: v_group(sch, 1))
                               for sch in range(SCH)]
                else:
                    relu_j[0] = mp.tile([P, 32, NW], bf16, tag="slotA",
                                        name="relu_j0")
                    fillers = offn_closures(0)
                fi = [0]

                def run_filler():
                    if fi[0] < len(fillers):
                        fillers[fi[0]]()
                        fi[0] += 1

                # deferred pair-end assembly: normalize + transpose of pair
                # p-1 issue after pair p's first score groups, so the PE
                # never waits on the DVE normalize chain.
                def assembly(pats, jj, hp):
                    at2 = [at2p.tile([P, P], bf16, tag="at2",
                                     name=f"at2_{q}") for q in range(4)]
                    for hs in range(2):
                        rec4 = smp.tile([P, 4, 1], f32, tag="rec4")
                        nc.vector.reciprocal(rec4, pats[hs][:, :, DK:DK + 1])
                        for q in range(4):
                            nc.vector.tensor_scalar_mul(
                                at2[q][:, hs * DK:(hs + 1) * DK],
                                pats[hs][:, q, 0:DK], rec4[:, q, :])
                    for q in range(4):
                        ptr = trp.tile([P, P], bf16, tag="tr")
                        nc.tensor.transpose(ptr, at2[q], ident_bf)
                        nc.vector.tensor_copy(
                            concat[jj][:, hp, q * P:(q + 1) * P], ptr)

                pending = [None]

                with nc.named_scope(f"attn{j}"):
                    for hp in range(DCH):
                        pats = [patp.tile([P, 4, DK + 1], f32,
                                          tag=f"pat{hs}", name=f"pat{hs}")
                                for hs in range(2)]
                        ets = {}

                        def sc_exp(g, hp=hp, j=j, ets=ets):
                            st = scp.tile([P, 2, NW], f32, tag="sc")
                            for hs in range(2):
                                p0 = hs * 64
                                nc.tensor.matmul(
                                    st[:, hs, :],
                                    ktb[p0:p0 + 64, hp, g * P:(g + 1) * P],
                                    qtb[p0:p0 + 64, 2 * hp + hs,
                                        j * NW:(j + 1) * NW],
                                    start=True, stop=True)
                            et = etp.tile([P, 2, NW], bf16, tag="et")
                            nc.scalar.activation(
                                et, st, AF.Exp, scale=float(1.0 / np.sqrt(DK)))
                            ets[g] = et

                        def a_v(g, hp=hp, pats=pats, ets=ets):
                            et = ets.pop(g)
                            for hs in range(2):
                                h = 2 * hp + hs
                                for q in range(4):
                                    nc.tensor.matmul(
                                        pats[hs][:, q, :],
                                        et[:, hs, q * P:(q + 1) * P],
                                        vaug[:, g, h * (DK + 1):
                                             (h + 1) * (DK + 1)],
                                        start=(g == 0 and q == 0),
                                        stop=(g == SCH - 1 and q == 3))

                        for gg in range(8):
                            sc_exp(2 * gg)
                            sc_exp(2 * gg + 1)
                            if gg == 0 and pending[0] is not None:
                                assembly(*pending[0])
                                pending[0] = None
                            if j == 1 or gg % 2 == 0:
                                run_filler()
                            if j == 1 and hp == 0:
                                run_filler()  # front-load O-j0 into pair 0
                            if gg >= 1:
                                a_v(2 * gg - 2)
                                a_v(2 * gg - 1)
                        a_v(14)
                        a_v(15)
                        pending[0] = (pats, j, hp)

                    assembly(*pending[0])
                    pending[0] = None

                if j == 1:
                    # preload tail's O weights before the drain closures
                    wo_c[0] = wload(wo, 0)
                    wo_c[1] = wload(wo, NW)

                # drain leftover fillers for this window
                with nc.named_scope(f"drain{j}"):
                    while fi[0] < len(fillers):
                        run_filler()

            # ================= tail: O/LN/FFN for j1 =================
            relu_j[1] = mp.tile([P, 32, NW], bf16, tag="slotC",
                                name="relu_j1")
            misc_state["tail"] = True
            with nc.named_scope("tail"):
                for c in offn_closures(1):
                    c()

    nc.compile()
    return nc


def _get_program():
    global _PROG
    if _PROG is None:
        _PROG = _build_program()
    return _PROG


def make_in_maps(x, Wq, bq, Wk, bk, Wv, bv, Wo, bo, alpha1, bias1, alpha2,
                 bias2, W1, b1, W2, b2):
    """Build the 8 per-core input maps. Shared arrays are reused by reference."""
    def b16(a):
        return np.ascontiguousarray(a).astype(BF16)

    shared = {
        "wq": b16(Wq), "wk": b16(Wk), "wv": b16(Wv), "wo": b16(Wo),
        "w1": b16(np.asarray(alpha1, np.float32)[:, None] * np.asarray(W1, np.float32)),
        "w2": b16(W2),
        "bq": np.asarray(bq, np.float32), "bk": np.asarray(bk, np.float32),
        "bvh": b16(bv),
        "b1": (np.asarray(b1, np.float32)
               + np.asarray(bias1, np.float32) @ np.asarray(W1, np.float32)),
        "alpha1": b16(alpha1),
        "beta1": b16(np.asarray(bias1, np.float32) + np.asarray(b2, np.float32)),
        "alpha2": b16(alpha2),
        "beta2": b16(bias2),
    }
    x = np.asarray(x, np.float32)
    bo = np.asarray(bo, np.float32)
    in_maps = []
    for c in range(NCORES):
        b, j = c // 2, c % 2
        xb = x[b]
        if j == 0:
            xt_np = xb.T
        else:
            xt_np = np.concatenate([xb[SQ:].T, xb[:SQ].T], axis=1)
        m = dict(shared)
        m["xt"] = b16(xt_np)
        m["xh"] = b16(xb[j * SQ:(j + 1) * SQ] + bo[None, :])
        in_maps.append(m)
    return in_maps


def kernel(**inputs):
    from concourse.bass_utils import run_bass_kernel_spmd

    nc = _get_program()
    in_maps = make_in_maps(**inputs)
    res = run_bass_kernel_spmd(nc, in_maps, core_ids=list(range(NCORES)))
    out = np.empty((B, S, D), np.float32)
    for c in range(NCORES):
        b, j = c // 2, c % 2
        out[b, j * SQ:(j + 1) * SQ, :] = res.results[c]["out"]
    return out
